# revision 9
# baseline (speedup 1.0000x reference)
"""Two-layer GraphSAGE on 8 Trainium2 NeuronCores.

Sharding: nodes row-sharded across the 8 cores (12,500 each, padded to
12,544 = 98*128); edges partitioned by destination owner so the
segment-sum is core-local; weight matrices replicated.

Per SAGE layer on each core:
  1. own activation shard cast to bf16, AllGather -> full 100,352-row
     gather table in local DRAM (3.2 MB/rank on the wire).
  2. dma_gather (int16-index gather ucode) pulls the per-edge source rows
     into SBUF in destination-sorted order.  The int16 index limit (32k)
     is handled by splitting the table into 4 row-range buckets and
     grouping each core's edge list by (window-group, bucket, window).
  3. segment-sum via one-hot matmuls: for each 128-destination window a
     PSUM tile accumulates onehot[e,dst]^T @ feat[e,f] over the window's
     edge blocks.  One-hots are built on DVE with is_equal against an
     iota row.
  4. mean (1/deg scale on ACT), transpose (PE), the two SAGE matmuls
     (aggregated + root), bias+ReLU on ACT, and a transpose back to
     row-major for the next layer's gather table.
"""

import math
import os
import sys

import numpy as np

for _p in ("/opt/trn_rl_repo", "/root/.axon_site/_ro/trn_rl_repo"):
    if os.path.isdir(_p) and _p not in sys.path:
        sys.path.append(_p)

import concourse.bass as bass
import concourse.bacc as bacc
import concourse.tile as tile
from concourse import mybir
from concourse.masks import make_identity

F32 = mybir.dt.float32
BF16 = mybir.dt.bfloat16
FP8 = mybir.dt.float8e4
I16 = mybir.dt.int16
P = 128
PAD_DLOC = 999.0  # one-hot compare target for padding edges -> all-zero row


class Cfg:
    def __init__(self, N=100000, E=1600000, C=8, d=128, n_cls=40,
                 WG=4, BUCKET=32768, MAX_IDX=1024, SINGLE_PACKET=False):
        assert N % C == 0
        self.N, self.E, self.C, self.d, self.n_cls = N, E, C, d, n_cls
        self.WG, self.BUCKET, self.MAX_IDX = WG, BUCKET, MAX_IDX
        self.SINGLE_PACKET = SINGLE_PACKET
        self.SH = N // C                       # nodes per core
        self.SHP = ((self.SH + P - 1) // P) * P  # padded shard rows
        self.W = self.SHP // P                 # dst windows per core
        self.TBL = C * self.SHP                # gather-table rows
        wpc = (self.W + 3) // 4               # windows per chunk (25)
        chw = [wpc, wpc, wpc, self.W - 3 * wpc]
        self.CHUNK_ROWS = [w * P for w in chw]  # local rows per chunk
        self.NBUK = 4
        self.NG = (self.W + WG - 1) // WG      # window groups
        assert d == P, "feature dim must be 128"


class Schedule:
    """Core-independent loop structure + per-core gather/one-hot data."""
    pass


def build_schedule(cfg: Cfg, src: np.ndarray, dst: np.ndarray,
                   deg: np.ndarray) -> Schedule:
    C, W, NBUK, WG, BUCKET = cfg.C, cfg.W, cfg.NBUK, cfg.WG, cfg.BUCKET
    SH, SHP = cfg.SH, cfg.SHP

    # chunk-major table layout: local rows are split into 4 window-aligned
    # chunks (25/25/25/23 windows); chunk k of every core is AllGathered
    # into its own Shared tensor, so chunk == bucket and the collective for
    # chunk k can fire as soon as each core has produced those windows.
    CH_ROWS = np.array(cfg.CHUNK_ROWS)         # local rows per chunk
    CH_LO = np.concatenate([[0], np.cumsum(CH_ROWS)[:-1]])   # local offsets
    CH_BASE = np.concatenate([[0], np.cumsum(CH_ROWS * C)[:-1]])  # table base
    owner = src // SH
    local = src - owner * SH
    ch = np.minimum(local // cfg.CHUNK_ROWS[0], cfg.NBUK - 1)
    trow = CH_BASE[ch] + owner * CH_ROWS[ch] + (local - CH_LO[ch])
    b_e = ch                                   # bucket of each edge
    dcore = dst // SH
    dloc = dst - dcore * SH
    w_e = dloc // P                            # dst window within the core
    dwin = (dloc % P).astype(np.float32)       # one-hot target

    key = ((dcore * W + w_e) * NBUK + b_e).astype(np.int64)
    cnt = np.bincount(key, minlength=C * W * NBUK).reshape(C, W, NBUK)

    # blocks per (window,bucket): identical across cores (SPMD program)
    tgt = cnt.max(axis=0).astype(np.int64)                     # [W, NBUK]
    M = (np.ceil(tgt / P)).astype(np.int64)                    # [W, NBUK]
    for w in range(W):                                         # >=1 block
        if M[w].sum() == 0:
            M[w, 0] = 1
            tgt[w, 0] = 1
    tgt = np.maximum(tgt, 1) * (M > 0)                         # reg >= 1
    s_tgt = tgt

    # stream order: window -> bucket -> block (a window's blocks are
    # contiguous across buckets, so DoubleRow pairs pack cross-bucket)
    s = Schedule()
    s.M = M
    s.tgt = s_tgt
    s.wruns = []                               # (w, w_blk0, tw, cells)
    blkoff = 0
    blk_of = np.zeros((W, NBUK), np.int64)     # global block idx of (w,b,0)
    for w in range(W):
        w0 = blkoff
        cells = []
        for b in range(NBUK):
            if M[w, b] > 0:
                blk_of[w, b] = blkoff
                cells.append((b, blkoff, int(M[w, b])))
                blkoff += int(M[w, b])
        s.wruns.append((w, w0, blkoff - w0, cells))
    s.B_tot = blkoff
    s.T_idx = s.B_tot * P                      # padded edge stream length

    # per-core data arrays
    order = np.argsort(key, kind="stable")
    off = np.zeros(C * W * NBUK + 1, np.int64)
    np.cumsum(cnt.ravel(), out=off[1:])
    rank = np.arange(cfg.E, dtype=np.int64) - off[key[order]]
    # stream position of each (sorted) edge
    base = (blk_of[w_e[order], b_e[order]] * P)
    pos = base + rank

    idx16 = np.zeros((C, 128, s.T_idx // 16), np.int16)
    dstloc = np.full((C, 128, s.B_tot), PAD_DLOC, np.float32)
    loc16 = (trow - CH_BASE[b_e]).astype(np.int16)
    # -1 idxs (trailing within a (w,b) call) are skipped by the gather
    # ucode; [cnt_c, tgt) positions stay 0 so num_idxs_reg is SPMD-uniform.
    base_flat = np.full(s.T_idx, -1, np.int16)
    for w in range(W):
        for b in range(NBUK):
            if M[w, b] > 0:
                b2 = blk_of[w, b] * P
                base_flat[b2:b2 + int(s_tgt[w, b])] = 0
    for c in range(C):
        m = dcore[order] == c
        p_c = pos[m]
        flat = base_flat.copy()
        flat[p_c] = loc16[order][m]
        wrapped = flat.reshape(-1, 16).T       # [16, T/16]
        idx16[c] = np.tile(wrapped, (8, 1))    # replicate for 8 Q7 cores
        dl = np.full(s.B_tot * P, PAD_DLOC, np.float32)
        dl[p_c] = dwin[order][m]
        dstloc[c] = dl.reshape(s.B_tot, P).T   # [128 lanes, B_tot blocks]
    s.idx16, s.dstloc = idx16, dstloc

    invdeg = 1.0 / np.maximum(deg, 1.0)
    inv = np.ones((C, 128, W), np.float32)
    for c in range(C):
        v = np.ones(SHP, np.float32)
        v[:SH] = invdeg[c * SH:(c + 1) * SH]
        inv[c] = v.reshape(W, P).T
    s.invdeg_t = inv
    return s


def build_program(cfg: Cfg, s: Schedule, debug: bool = False):
    """Emit the SPMD Bass program (identical on all 8 cores)."""
    C, W, NBUK, NCLS = cfg.C, cfg.W, cfg.NBUK, cfg.n_cls
    SHP, TBL, BUCKET = cfg.SHP, cfg.TBL, cfg.BUCKET

    nc = bacc.Bacc("TRN2", target_bir_lowering=False, debug=debug,
                   num_devices=C, num_swdge_queues=4,
                   dynamic_dma_scratch_size=98304)

    x_own = nc.dram_tensor("x_own", [SHP, P], F32, kind="ExternalInput")
    idx_in = nc.dram_tensor("idx16", [128, s.T_idx // 16], I16,
                            kind="ExternalInput")
    dloc_in = nc.dram_tensor("dstloc", [128, s.B_tot], F32,
                             kind="ExternalInput")
    inv_in = nc.dram_tensor("invdeg", [128, W], F32, kind="ExternalInput")
    iota_in = nc.dram_tensor("iota", [128, 128], F32, kind="ExternalInput")
    w_ins = {}
    for nm, shp in (("wl1t", [P, P]), ("wr1t", [P, P]),
                    ("wl2t", [P, NCLS]), ("wr2t", [P, NCLS])):
        w_ins[nm] = nc.dram_tensor(nm, shp, F32, kind="ExternalInput")
    bl1_in = nc.dram_tensor("bl1", [P, 1], F32, kind="ExternalInput")
    bl2_in = nc.dram_tensor("bl2", [NCLS, 1], F32, kind="ExternalInput")
    out_d = nc.dram_tensor("out", [SHP, NCLS], F32, kind="ExternalOutput")

    x_in_d = nc.dram_tensor("x_bf_own", [SHP, P], BF16)
    h_in_d = nc.dram_tensor("h_bf_own", [SHP, P], BF16)
    x_full = [nc.dram_tensor(f"x_full{k}", [C * cfg.CHUNK_ROWS[k], P], BF16,
                             addr_space="Shared") for k in range(4)]
    h_full = [nc.dram_tensor(f"h_full{k}", [C * cfg.CHUNK_ROWS[k], P], BF16,
                             addr_space="Shared") for k in range(4)]

    rg = [list(range(C))]

    with tile.TileContext(nc) as tc:
        cpool = tc.alloc_tile_pool(name="consts", bufs=1)
        stage = tc.alloc_tile_pool(name="stage", bufs=2)

        ident_b = cpool.tile([P, P], BF16)
        make_identity(nc, ident_b[:])
        ident_f = cpool.tile([P, P], F32)
        make_identity(nc, ident_f[:])

        iota_f = cpool.tile([128, 128], F32)
        nc.sync.dma_start(out=iota_f[:], in_=iota_in[:])
        iota_b = cpool.tile([128, 128], BF16)
        nc.vector.tensor_copy(out=iota_b[:], in_=iota_f[:])

        wt = {}
        for nm in ("wl1t", "wr1t", "wl2t", "wr2t"):
            shp = [P, P] if nm in ("wl1t", "wr1t") else [P, NCLS]
            st = stage.tile(shp, F32, tag="wstage")
            nc.sync.dma_start(out=st[:], in_=w_ins[nm][:])
            wt[nm] = cpool.tile(shp, BF16, name=f"w_{nm}")
            nc.vector.tensor_copy(out=wt[nm][:], in_=st[:])
        bl1_t = cpool.tile([P, 1], F32)
        nc.sync.dma_start(out=bl1_t[:], in_=bl1_in[:])
        bl2_t = cpool.tile([NCLS, 1], F32)
        nc.sync.dma_start(out=bl2_t[:], in_=bl2_in[:])
        inv_t = cpool.tile([128, W], F32)
        nc.sync.dma_start(out=inv_t[:], in_=inv_in[:])
        dloc_f = stage.tile([128, s.B_tot], F32, tag="dlocf", bufs=1)
        nc.sync.dma_start(out=dloc_f[:], in_=dloc_in[:])
        dloc_sb = cpool.tile([128, s.B_tot], BF16)
        nc.vector.tensor_copy(out=dloc_sb[:], in_=dloc_f[:])

        xT = cpool.tile([P, SHP], BF16)        # x_own^T, bf16
        hT = cpool.tile([P, SHP], BF16)        # h_own^T, bf16

        # ---- phase 0: cast x to bf16 (row major for the table, transposed
        # for the dense term), then AllGather the table.
        with tc.tile_pool(name="ph0", bufs=3) as ph0, \
             tc.tile_pool(name="ph0p", bufs=2, space="PSUM") as ph0p:
            WB = 8
            for wb in range(0, W, WB):
                nw = min(WB, W - wb)
                r0 = wb * P
                xrow_f = ph0.tile([P, nw, P], F32, tag="xf")
                nc.sync.dma_start(
                    out=xrow_f[:],
                    in_=x_own[r0:r0 + nw * P, :].rearrange(
                        "(a p) f -> p a f", p=P))
                xrow_b = ph0.tile([P, nw, P], BF16, tag="xb")
                nc.vector.tensor_copy(out=xrow_b[:], in_=xrow_f[:])
                nc.sync.dma_start(
                    out=x_in_d[r0:r0 + nw * P, :].rearrange(
                        "(a p) f -> p a f", p=P),
                    in_=xrow_b[:])
                for a in range(nw):
                    pt = ph0p.tile([P, P], BF16, tag="pt")
                    nc.tensor.transpose(out=pt[:], in_=xrow_b[:, a, :],
                                        identity=ident_b[:])
                    nc.vector.tensor_copy(
                        out=xT[:, (wb + a) * P:(wb + a + 1) * P], in_=pt[:])

        off = 0
        for k in range(4):
            nc.gpsimd.collective_compute(
                "AllGather", mybir.AluOpType.bypass, replica_groups=rg,
                ins=[x_in_d[off:off + cfg.CHUNK_ROWS[k], :]],
                outs=[x_full[k][:]])
            off += cfg.CHUNK_ROWS[k]

        qctr = [0]

        def sage_layer(table, dense_rhs, wl, wr, bias_t, relu, m_out, out_sink):
            """One SAGE conv over the edge schedule.

            m_out: output feature count (P for layer 1, NCLS for layer 2)
            out_sink(w, psum_ap): consumes the [m_out, 128] transposed
            output window (post bias/activation).
            """
            gp = tc.alloc_tile_pool(name="gath", bufs=4)
            g8p = tc.alloc_tile_pool(name="g8", bufs=4)
            ohp = tc.alloc_tile_pool(name="oh", bufs=4)
            ixp = tc.alloc_tile_pool(name="ixp", bufs=4)
            max_nblk = max(tw for _, _, tw, _ in s.wruns)
            for _ in range(4):
                twm = gp.tile([128, max_nblk, P], BF16, tag="g")
                nc.vector.memset(twm[:], 0.0)
            ap_ = tc.alloc_tile_pool(name="psA", bufs=cfg.WG, space="PSUM")
            ep_ = tc.alloc_tile_pool(name="psE", bufs=1, space="PSUM")
            sb_ = tc.alloc_tile_pool(name="esb", bufs=3)
            for w, w0, tw, cells in s.wruns:
                ixt = ixp.tile([128, tw * P // 16], I16, tag="ix")
                nc.sync.dma_start(
                    out=ixt[:],
                    in_=idx_in[:, w0 * P // 16:(w0 + tw) * P // 16])
                gt = gp.tile([128, tw, P], BF16, tag="g")
                gt8 = g8p.tile([128, tw, P], FP8, tag="g8")
                for b, cb, m in cells:
                    cell_tgt = int(s.tgt[w, b])
                    for c0 in range(0, m, cfg.MAX_IDX // P):
                        cn = min(cfg.MAX_IDX // P, m - c0)
                        i0 = (cb - w0 + c0) * P // 16
                        reg = min(cn * P, cell_tgt - c0 * P)
                        nc.gpsimd.dma_gather(
                            out_ap=gt[:, cb - w0 + c0:cb - w0 + c0 + cn, :],
                            in_ap=table[b][:],
                            idxs_ap=ixt[:, i0:i0 + cn * P // 16],
                            num_idxs=cn * P,
                            num_idxs_reg=reg,
                            elem_size=P,
                            single_packet=cfg.SINGLE_PACKET,
                            queue_num=qctr[0] % 4)
                        qctr[0] += 1
                    nc.scalar.mul(gt8[:, cb - w0:cb - w0 + m, :],
                                  gt[:, cb - w0:cb - w0 + m, :], 1.0)
                oht = ohp.tile([128, tw, P], FP8, tag="oh", name=f"oh_{w}")
                nc.vector.tensor_tensor(
                    out=oht[:],
                    in0=iota_b[:].rearrange(
                        "p (o n) -> p o n", o=1).to_broadcast(
                        [128, tw, P]),
                    in1=dloc_sb[:, w0:w0 + tw].rearrange(
                        "p (n o) -> p n o", o=1).to_broadcast(
                        [128, tw, P]),
                    op=mybir.AluOpType.is_equal)
                psA = ap_.tile([P, P], F32, tag="A", name=f"psA_{w}")
                j = 0
                while j < tw:
                    nb = 2 if j + 1 < tw else 1
                    if nb == 2:
                        nc.tensor.matmul(
                            psA[:], lhsT=oht[:, j:j + 2, :],
                            rhs=gt8[:, j:j + 2, :],
                            start=(j == 0), stop=(j + 2 == tw),
                            perf_mode=mybir.MatmulPerfMode.DoubleRow)
                    else:
                        nc.tensor.matmul(
                            psA[:], lhsT=oht[:, j, :],
                            rhs=gt8[:, j, :],
                            start=(j == 0), stop=(j + 1 == tw))
                    j += nb
                wc = w * P
                agg = sb_.tile([P, P], BF16, tag="agg")
                nc.scalar.mul(agg[:], psA[:], inv_t[:, w:w + 1])
                pt = ep_.tile([P, P], BF16, tag="T")
                nc.tensor.transpose(out=pt[:], in_=agg[:],
                                    identity=ident_b[:])
                aggT = sb_.tile([P, P], BF16, tag="aggT")
                nc.vector.tensor_copy(out=aggT[:], in_=pt[:])
                pb = ep_.tile([m_out, P], F32, tag="B")
                nc.tensor.matmul(pb[:], lhsT=wl[:], rhs=aggT[:],
                                 start=True, stop=False)
                nc.tensor.matmul(pb[:], lhsT=wr[:],
                                 rhs=dense_rhs[:, wc:wc + P],
                                 start=False, stop=True)
                out_sink(w, pb, bias_t)
            for pool in (sb_, ep_, ap_, ixp, ohp, g8p, gp):
                pool.release()

        # ---- layer 1 ----
        with tc.tile_pool(name="l1o", bufs=2) as l1o, \
             tc.tile_pool(name="l1p", bufs=2, space="PSUM") as l1p:
            def sink1(w, pb, bias_t):
                wc = w * P
                nc.scalar.activation(hT[:, wc:wc + P], pb[:],
                                     mybir.ActivationFunctionType.Relu,
                                     bias=bias_t[:], scale=1.0)
                pc = l1p.tile([P, P], BF16, tag="C")
                nc.tensor.transpose(out=pc[:], in_=hT[:, wc:wc + P],
                                    identity=ident_b[:])
                hrow = l1o.tile([P, P], BF16, tag="hrow")
                nc.vector.tensor_copy(out=hrow[:], in_=pc[:])
                nc.sync.dma_start(out=h_in_d[wc:wc + P, :], in_=hrow[:])

            sage_layer(x_full, xT, wt["wl1t"], wt["wr1t"], bl1_t,
                       relu=True, m_out=P, out_sink=sink1)

        off = 0
        for k in range(4):
            nc.gpsimd.collective_compute(
                "AllGather", mybir.AluOpType.bypass, replica_groups=rg,
                ins=[h_in_d[off:off + cfg.CHUNK_ROWS[k], :]],
                outs=[h_full[k][:]])
            off += cfg.CHUNK_ROWS[k]

        # ---- layer 2 ----
        with tc.tile_pool(name="l2o", bufs=2) as l2o, \
             tc.tile_pool(name="l2p", bufs=2, space="PSUM") as l2p:
            def sink2(w, pb, bias_t):
                wc = w * P
                oT = l2o.tile([NCLS, P], F32, tag="oT")
                nc.scalar.activation(oT[:], pb[:],
                                     mybir.ActivationFunctionType.Identity,
                                     bias=bias_t[:], scale=1.0)
                pc = l2p.tile([P, NCLS], F32, tag="C2")
                nc.tensor.matmul(pc[:], lhsT=oT[:], rhs=ident_f[:NCLS, :NCLS],
                                 is_transpose=True)
                orow = l2o.tile([P, NCLS], F32, tag="orow")
                nc.vector.tensor_copy(out=orow[:], in_=pc[:])
                nc.sync.dma_start(out=out_d[wc:wc + P, :], in_=orow[:])

            sage_layer(h_full, hT, wt["wl2t"], wt["wr2t"], bl2_t,
                       relu=False, m_out=NCLS, out_sink=sink2)

        for pool in (stage, cpool):
            pool.release()

    nc.compile()
    return nc


def make_inputs(cfg: Cfg, s: Schedule, x, Wl1, bl1, Wr1, Wl2, bl2, Wr2):
    """Per-core in_maps for run_bass_kernel_spmd."""
    C, SH, SHP, W, NCLS = cfg.C, cfg.SH, cfg.SHP, cfg.W, cfg.n_cls
    iota = np.tile(np.arange(128, dtype=np.float32), (128, 1))
    maps = []
    for c in range(C):
        xo = np.zeros((SHP, P), np.float32)
        xo[:SH] = x[c * SH:(c + 1) * SH]
        maps.append({
            "x_own": xo,
            "idx16": s.idx16[c],
            "dstloc": s.dstloc[c],
            "invdeg": s.invdeg_t[c],
            "iota": iota,
            "wl1t": np.ascontiguousarray(Wl1.T.astype(np.float32)),
            "wr1t": np.ascontiguousarray(Wr1.T.astype(np.float32)),
            "wl2t": np.ascontiguousarray(Wl2.T.astype(np.float32)),
            "wr2t": np.ascontiguousarray(Wr2.T.astype(np.float32)),
            "bl1": bl1.astype(np.float32).reshape(P, 1),
            "bl2": bl2.astype(np.float32).reshape(NCLS, 1),
        })
    return maps


def prepare(cfg: Cfg, x, edge_index, Wl1, bl1, Wr1, Wl2, bl2, Wr2):
    x = np.asarray(x, np.float32)
    ei = np.asarray(edge_index, np.int64)
    src, dst = ei[0], ei[1]
    deg = np.bincount(dst, minlength=cfg.N).astype(np.float32)
    s = build_schedule(cfg, src, dst, deg)
    maps = make_inputs(cfg, s, x, Wl1, bl1, Wr1, Wl2, bl2, Wr2)
    return s, maps


def run(x, edge_index, Wl1, bl1, Wr1, Wl2, bl2, Wr2, cfg=None, **spmd_kwargs):
    from concourse.bass_utils import run_bass_kernel_spmd
    cfg = cfg or Cfg()
    s, maps = prepare(cfg, x, edge_index, Wl1, bl1, Wr1, Wl2, bl2, Wr2)
    nc = build_program(cfg, s)
    res = run_bass_kernel_spmd(nc, maps, core_ids=list(range(cfg.C)),
                               **spmd_kwargs)
    out = np.concatenate([res.results[c]["out"][:cfg.SH]
                          for c in range(cfg.C)], axis=0)
    return out.astype(np.float32), res


def kernel(x, edge_index, Wl1, bl1, Wr1, Wl2, bl2, Wr2):
    out, _ = run(x, edge_index, Wl1, bl1, Wr1, Wl2, bl2, Wr2)
    return out

